# revision 1
# baseline (speedup 1.0000x reference)
"""TRN2 Bass kernel: ClapAudio window self-attention (B=2048 windows of 64
tokens, C=256, 8 heads x d=32), data-parallel over windows across 8 cores.

Host side: shards hidden_states, precomputes EB = exp(rel-pos-bias + mask)^T
(folding both additive score biases into one multiplicative table applied
after exp), and passes transposed weights in bf16.

Device side (per core, 256 windows): see build() docstring.
"""

import numpy as np
import ml_dtypes

import concourse.bass as bass
import concourse.mybir as mybir
import concourse.tile as tile
from concourse.bass_utils import run_bass_kernel_spmd
from concourse.masks import make_identity

DT = mybir.dt
F32 = DT.float32
BF16 = DT.bfloat16

N_CORES = 8
B = 2048
C = 256
H = 8
D = 32
WINTOK = 64
SCALE = 1.0 / np.sqrt(np.float32(D))

def _wait_cap(inst):
    """Max sem waits the walrus encoding of this instruction tolerates."""
    if isinstance(inst, (mybir.InstDrain, mybir.InstNoOp)):
        return 1  # CTRL_NO_STRUCT
    if isinstance(inst, (mybir.InstDMACopy, mybir.InstDMA, mybir.InstDmaTransposeAnt)):
        return 1  # PSEUDO_DMA_DIRECT2D
    return 1


def split_drain_waits(nc):
    """Walrus instruction encodings only fit a limited number of sem waits;
    Tile can attach more. Hoist excess waits onto NoOps inserted before the
    instruction on the same engine (in-order sequencers make this
    equivalent, if slightly more conservative)."""
    for f in nc.m.functions:
        for bb in f.blocks:
            new_insts = []
            for inst in bb.instructions:
                si = inst.sync_info
                cap = _wait_cap(inst)
                if si is not None and si.on_wait and len(si.on_wait) > cap:
                    waits = list(si.on_wait)
                    keep, rest = waits[:cap], waits[cap:]
                    for i in range(0, len(rest), 1):
                        new_insts.append(
                            mybir.InstNoOp(
                                name=f"{inst.name}-waitsplit-{i}",
                                engine=inst.engine,
                                sync_info=mybir.SyncInfo(
                                    on_wait=[rest[i]], on_update=[]
                                ),
                            )
                        )
                    inst.sync_info = mybir.SyncInfo(
                        on_wait=keep, on_update=list(si.on_update or [])
                    )
                new_insts.append(inst)
            bb.instructions[:] = new_insts


def build(n_windows=256, chunk_windows=8, split_waits=True):
    """Emit the per-core kernel.

    Layouts:
      x    DRAM [ntok, 256] bf16
      eb   DRAM [32, 128, 512] bf16:
           eb[t, slot*64+k, 64h+q] = exp(rpb[h,q,k] + mask[2t+slot,q,k])
      wqt/wkt/wvt DRAM [256, 256] bf16 = W.T  ([C_in, C_out])
      bqv/bkv DRAM [256] f32 ; bvr DRAM [1, 256] bf16
      out  DRAM [ntok, 256] f32

    Per 512-token chunk (8 windows = 4 window pairs): DMA X in; X^T via PE
    transpose; weight-stationary Q^T/K^T projections (+bias on copy);
    X^T-stationary V projection (+bv via K=1 ones matmul) scattered into a
    block-diag augmented V (with ones cols for softmax sums); per
    window-head scores^T = K^T.T @ Q^T (K=d=32, tile_position round-robin);
    ACT exp (scale folded, no max-subtraction needed at these magnitudes);
    GPSIMD multiply by resident EB table; per-head pair-matmul
    probs.T @ V_aug giving unnormalized ctx + softmax sums; DVE reciprocal +
    broadcast-AP multiply to normalize straight into the f32 staging tile;
    2 DMAs out per window pair.
    """
    assert n_windows % chunk_windows == 0 and chunk_windows % 2 == 0
    ntok = n_windows * WINTOK
    n_chunks = n_windows // chunk_windows
    chunk_tok = chunk_windows * WINTOK  # 512
    n_tile = chunk_tok // 128  # 4 token-tiles (each = 1 window pair)
    assert chunk_tok == 512

    nc = bass.Bass()
    x = nc.declare_dram_parameter("x", [ntok, C], BF16, isOutput=False)
    eb = nc.declare_dram_parameter("eb", [32, 128, 512], BF16, isOutput=False)
    wqt = nc.declare_dram_parameter("wqt", [C, C], BF16, isOutput=False)
    wkt = nc.declare_dram_parameter("wkt", [C, C], BF16, isOutput=False)
    wvt = nc.declare_dram_parameter("wvt", [C, C], BF16, isOutput=False)
    bqv = nc.declare_dram_parameter("bqv", [C], F32, isOutput=False)
    bkv = nc.declare_dram_parameter("bkv", [C], F32, isOutput=False)
    bvr = nc.declare_dram_parameter("bvr", [128, C], BF16, isOutput=False)
    out = nc.declare_dram_parameter("out", [ntok, C], F32, isOutput=True)

    with tile.TileContext(nc) as tc:
        with (
            tc.tile_pool(name="const", bufs=1) as cpool,
            tc.tile_pool(name="acts", bufs=2) as apool,
            tc.tile_pool(name="probs", bufs=3) as ppool,
            tc.tile_pool(name="stage", bufs=3) as spool,
            tc.tile_pool(name="small", bufs=3) as smpool,
            tc.tile_pool(name="vv", bufs=3) as vpool,
            tc.tile_pool(name="pp", bufs=3, space="PSUM") as pp,
            tc.tile_pool(name="psc", bufs=3, space="PSUM") as psc,
            tc.tile_pool(name="pctx", bufs=2, space="PSUM") as pctx,
        ):
            # ---- constants ----
            eb_sb = cpool.tile([128, 32 * 512], BF16)
            for t in range(32):
                nc.sync.dma_start(eb_sb[:, t * 512 : (t + 1) * 512], eb[t, :, :])
            wq_sb = cpool.tile([128, 512], BF16)
            wk_sb = cpool.tile([128, 512], BF16)
            wv_sb = cpool.tile([128, 512], BF16)
            for w_sb, w_dram in ((wq_sb, wqt), (wk_sb, wkt), (wv_sb, wvt)):
                for ck in range(2):
                    nc.sync.dma_start(
                        w_sb[:, ck * 256 : (ck + 1) * 256],
                        w_dram[ck * 128 : (ck + 1) * 128, :],
                    )
            bq_sb = cpool.tile([128, 2], F32)
            bk_sb = cpool.tile([128, 2], F32)
            nc.sync.dma_start(bq_sb[:], bqv.rearrange("(m p) -> p m", p=128))
            nc.sync.dma_start(bk_sb[:], bkv.rearrange("(m p) -> p m", p=128))
            # bv as K=128 constant matmul: lhsT = 1/128, rhs = bv broadcast
            inv_sb = cpool.tile([128, 128], BF16)
            nc.vector.memset(inv_sb[:], 1.0 / 128.0)
            bvb_sb = cpool.tile([128, C], BF16)
            nc.sync.dma_start(bvb_sb[:], bvr[:, :])

            # static vaugs: [128, 264] per window, double-buffered (4 tiles).
            # rows 0:64 pair-block j cols 66j+dv = V_win[:, head 2j]; ones at
            # 66j+32. rows 64:128 cols 66j+33+dv = V_win[:, head 2j+1]; ones
            # at 66j+65.
            vaugs = []
            for vb in range(4):
                va = cpool.tile([128, 264], BF16, tag=f"vaug{vb}")
                nc.vector.memset(va[:], 0.0)
                nc.vector.memset(
                    va[0:64, :].rearrange("p (j c) -> p j c", j=4)[:, :, 32:33], 1.0
                )
                nc.vector.memset(
                    va[64:128, :].rearrange("p (j c) -> p j c", j=4)[:, :, 65:66], 1.0
                )
                vaugs.append(va)

            # ---- main loop over chunks ----
            for u in range(n_chunks):
                t0 = u * chunk_tok

                # X^T loaded directly via DMA xbar transpose (bf16)
                xt_sb = apool.tile([128, 2 * 512], BF16, tag="xt")
                for ck in range(2):
                    nc.sync.dma_start_transpose(
                        xt_sb[:, ck * 512 : (ck + 1) * 512],
                        x[t0 : t0 + chunk_tok, ck * 128 : (ck + 1) * 128],
                    )

                # Q^T / K^T projections (weights stationary)
                qt_sb = apool.tile([128, 2 * 512], BF16, tag="qt")
                kt_sb = apool.tile([128, 2 * 512], BF16, tag="kt")
                for qk, (w_sb, b_sb, dst) in enumerate(
                    ((wq_sb, bq_sb, qt_sb), (wk_sb, bk_sb, kt_sb))
                ):
                    for m in range(2):
                        prj = pp.tile([128, 512], F32, tag="pp")
                        for ck in range(2):
                            nc.tensor.matmul(
                                prj[:],
                                w_sb[:, ck * 256 + m * 128 : ck * 256 + (m + 1) * 128],
                                xt_sb[:, ck * 512 : (ck + 1) * 512],
                                start=(ck == 0),
                                stop=(ck == 1),
                            )
                        cp_out = dst[:, m * 512 : (m + 1) * 512]
                        if qk == 0:
                            nc.scalar.activation(
                                cp_out,
                                prj[:],
                                mybir.ActivationFunctionType.Identity,
                                bias=b_sb[:, m : m + 1],
                            )
                        else:
                            nc.vector.tensor_scalar_add(
                                cp_out, prj[:], b_sb[:, m : m + 1]
                            )

                # half-shift copies: heads h%4 in {2,3} live at rows 64:128;
                # move them to rows 0:64 so every K=32 matmul uses strips 0/32
                qt2_sb = apool.tile([64, 2 * 512], BF16, tag="qt2")
                kt2_sb = apool.tile([64, 2 * 512], BF16, tag="kt2")
                nc.sync.dma_start(qt2_sb[:], qt_sb[64:128, :])
                nc.sync.dma_start(kt2_sb[:], kt_sb[64:128, :])

                def qk_slice(src, src2, h, cols):
                    r = h % 4
                    m = h // 4
                    if r < 2:
                        return src[32 * r : 32 * r + 32, m * 512 + cols[0] : m * 512 + cols[1]]
                    return src2[32 * (r - 2) : 32 * (r - 2) + 32, m * 512 + cols[0] : m * 512 + cols[1]]

                # V projection + bv (whole chunk), then one swap DMA pair
                vtmp = vpool.tile([128, n_tile * 256], BF16, tag="vt")
                for i in range(n_tile):
                    vps = pp.tile([128, 256], F32, tag="pp")
                    for ck in range(2):
                        nc.tensor.matmul(
                            vps[:],
                            xt_sb[:, ck * 512 + i * 128 : ck * 512 + (i + 1) * 128],
                            wv_sb[:, ck * 256 : (ck + 1) * 256],
                            start=(ck == 0),
                            stop=False,
                        )
                    nc.tensor.matmul(
                        vps[:], inv_sb[:, :], bvb_sb[:, :], start=False, stop=True
                    )
                    nc.vector.tensor_copy(
                        vtmp[:, i * 256 : (i + 1) * 256], vps[:]
                    )
                vswap = vpool.tile([128, n_tile * 256], BF16, tag="vs")
                nc.sync.dma_start(vswap[64:128, :], vtmp[0:64, :])
                nc.sync.dma_start(vswap[0:64, :], vtmp[64:128, :])

                stgc = spool.tile([128, n_tile * 256], F32, tag="st")
                for i in range(n_tile):
                    va_a = vaugs[2 * (i % 2)]
                    va_b = vaugs[2 * (i % 2) + 1]
                    # vaug_A: top = V(wA) even heads, bottom = V(wA) odd heads
                    nc.vector.tensor_copy(
                        va_a[0:64, :].rearrange("p (j c) -> p j c", j=4)[:, :, 0:32],
                        vtmp[0:64, i * 256 : (i + 1) * 256].rearrange(
                            "p (j c) -> p j c", j=4
                        )[:, :, 0:32],
                    )
                    nc.scalar.copy(
                        va_a[64:128, :].rearrange("p (j c) -> p j c", j=4)[:, :, 33:65],
                        vswap[64:128, i * 256 : (i + 1) * 256].rearrange(
                            "p (j c) -> p j c", j=4
                        )[:, :, 32:64],
                    )
                    # vaug_B: top = V(wB) even heads, bottom = V(wB) odd heads
                    nc.vector.tensor_copy(
                        va_b[0:64, :].rearrange("p (j c) -> p j c", j=4)[:, :, 0:32],
                        vswap[0:64, i * 256 : (i + 1) * 256].rearrange(
                            "p (j c) -> p j c", j=4
                        )[:, :, 0:32],
                    )
                    nc.scalar.copy(
                        va_b[64:128, :].rearrange("p (j c) -> p j c", j=4)[:, :, 33:65],
                        vtmp[64:128, i * 256 : (i + 1) * 256].rearrange(
                            "p (j c) -> p j c", j=4
                        )[:, :, 32:64],
                    )

                    # ---- attention for this window pair ----
                    wp = u * n_tile + i
                    nwp = wp % 32
                    scp = psc.tile([128, 512], F32, tag="sc")
                    for h in range(H):
                        s = 32 * (h % 2)
                        for win in range(2):
                            cols = (i * 128 + win * 64, i * 128 + win * 64 + 64)
                            b = (h // 2) * 2 + win
                            nc.tensor.matmul(
                                scp[s * 2 : s * 2 + 64, b * 64 : b * 64 + 64],
                                qk_slice(kt_sb, kt2_sb, h, cols),
                                qk_slice(qt_sb, qt2_sb, h, cols),
                                start=True,
                                stop=True,
                                tile_position=(s, s * 2),
                            )
                    probs = ppool.tile([128, 512], BF16, tag="pr")
                    nc.scalar.activation(
                        probs[:],
                        scp[:],
                        mybir.ActivationFunctionType.Exp,
                        scale=float(SCALE),
                    )
                    ebs = eb_sb[:, nwp * 512 : (nwp + 1) * 512]
                    if i % 2 == 0:
                        nc.gpsimd.tensor_mul(probs[:], probs[:], ebs)
                    else:
                        nc.vector.tensor_mul(probs[:], probs[:], ebs)

                    ctxp = pctx.tile([128, 264], F32, tag="ctx")
                    for j in range(4):
                        for win in range(2):
                            va = va_a if win == 0 else va_b
                            nc.tensor.matmul(
                                ctxp[win * 64 : win * 64 + 64, j * 66 : j * 66 + 66],
                                probs[:, (j * 2 + win) * 64 : (j * 2 + win) * 64 + 64],
                                va[:, j * 66 : j * 66 + 66],
                                start=True,
                                stop=True,
                                tile_position=(0, 64 * win),
                            )

                    recips = smpool.tile([128, 8], F32, tag="rc")
                    sums_ap = ctxp.rearrange("p (j par c) -> p j par c", j=4, par=2)[
                        :, :, :, 32:33
                    ]
                    nc.vector.reciprocal(recips[:], sums_ap)

                    ctx_ap = ctxp.rearrange("p (j par c) -> p j par c", j=4, par=2)[
                        :, :, :, 0:32
                    ]
                    rec_ap = recips.rearrange("p (j par one) -> p j par one", j=4, one=1)
                    ctx_b, rec_b = bass.broadcast_tensor_aps(ctx_ap, rec_ap)
                    out_ap = stgc[:, i * 256 : (i + 1) * 256].rearrange(
                        "p (j par c) -> p j par c", j=4, par=2
                    )
                    nc.vector.tensor_tensor(out_ap, ctx_b, rec_b, mybir.AluOpType.mult)

                nc.sync.dma_start(
                    out[t0 : t0 + chunk_tok, :].rearrange("(i p) c -> p i c", p=128),
                    stgc.rearrange("p (i c) -> p i c", i=n_tile),
                )

    if split_waits:
        split_drain_waits(nc)
    return nc


def ref_shard(x, eb_full, wq, bq, wk, bk, wv, bv):
    """NumPy reference for one shard (dev-time check)."""
    ntok = x.shape[0]
    nwin = ntok // WINTOK
    q = x @ wq.T + bq
    k = x @ wk.T + bk
    v = x @ wv.T + bv
    outp = np.zeros((ntok, C), np.float32)
    for w in range(nwin):
        t = slice(w * WINTOK, (w + 1) * WINTOK)
        nwp, slot = (w // 2) % 32, w % 2
        for h in range(H):
            qh = q[t, h * D : (h + 1) * D]
            kh = k[t, h * D : (h + 1) * D]
            vh = v[t, h * D : (h + 1) * D]
            sT = (kh @ qh.T) * SCALE
            b = (h // 2) * 2 + slot
            ebm = eb_full[nwp, (h % 2) * 64 : (h % 2) * 64 + 64, b * 64 : b * 64 + 64]
            pu = np.exp(sT) * ebm
            ctx = pu.T @ vh
            s = pu.sum(axis=0)
            outp[t, h * D : (h + 1) * D] = ctx / s[:, None]
    return outp


_NC_CACHE = {}


def _get_nc():
    key = "main"
    if key not in _NC_CACHE:
        _NC_CACHE[key] = build(n_windows=B // N_CORES)
    return _NC_CACHE[key]


def _pack_eb(bias_table, rel_index, attention_mask):
    # rpb[h, q, k] = bias_table[rel_index[q, k], h]
    rpb = bias_table[rel_index.reshape(-1)].reshape(64, 64, H).transpose(2, 0, 1)
    e = np.exp(
        rpb[None].astype(np.float64) + attention_mask[:, None].astype(np.float64)
    ).astype(np.float32)
    # e [nw, h, q, k] -> eb[t, (h%2)*64 + k, ((h//2)*2 + slot)*64 + q]
    e2 = e.transpose(0, 1, 3, 2)  # [nw, h, k, q]
    e3 = e2.reshape(32, 2, 4, 2, 64, 64)  # [t, slot, j, par, k, q]
    e4 = e3.transpose(0, 3, 4, 2, 1, 5)  # [t, par, k, j, slot, q]
    return np.ascontiguousarray(e4.reshape(32, 128, 512))


def kernel(
    hidden_states,
    attention_mask,
    Wq,
    bq,
    Wk,
    bk,
    Wv,
    bv,
    bias_table,
    rel_index,
):
    bf = ml_dtypes.bfloat16
    nc = _get_nc()

    xs = np.ascontiguousarray(hidden_states.reshape(B * WINTOK, C)).astype(bf)
    eb = _pack_eb(
        np.asarray(bias_table, np.float32),
        np.asarray(rel_index),
        np.asarray(attention_mask, np.float32),
    ).astype(bf)
    common = {
        "eb": eb,
        "wqt": np.ascontiguousarray(Wq.T).astype(bf),
        "wkt": np.ascontiguousarray(Wk.T).astype(bf),
        "wvt": np.ascontiguousarray(Wv.T).astype(bf),
        "bqv": np.asarray(bq, np.float32),
        "bkv": np.asarray(bk, np.float32),
        "bvr": np.tile(np.asarray(bv, np.float32)[None, :], (128, 1)).astype(bf),
    }
    shard_tok = (B // N_CORES) * WINTOK
    in_maps = [
        {"x": xs[c * shard_tok : (c + 1) * shard_tok], **common}
        for c in range(N_CORES)
    ]
    res = run_bass_kernel_spmd(nc, in_maps, list(range(N_CORES)))
    outp = np.concatenate(
        [res.results[c]["out"] for c in range(N_CORES)], axis=0
    )
    return outp.reshape(B, WINTOK, C).astype(np.float32)



# revision 4
# speedup vs baseline: 1.1043x; 1.1043x over previous
"""TRN2 Bass kernel: ClapAudio window self-attention (B=2048 windows of 64
tokens, C=256, 8 heads x d=32), data-parallel over windows across 8 cores.

Host side: shards + pre-transposes hidden_states (xt [C, ntok]), precomputes
EB = exp(rel-pos-bias + mask)^T (folding both additive score biases into one
multiplicative table applied after exp), passes transposed weights in bf16.
Output returned bf16 from device, cast to f32 on host.

Device side (per core, 256 windows, 32 chunks of 8 windows):
  - xt chunk loaded with 2 plain DMAs (host already transposed).
  - Q^T/K^T projections, weights stationary; bias folded into the
    PSUM->SBUF copy (ACT for Q, DVE for K).
  - qbd: block-diagonal Q operand [128, 2*8*256] built by 4 SB->SB DMAs
    (band r of Q^T lands in col-block hh=r; zeros elsewhere are static).
    Scores then take 16 matmuls/chunk of [K=128, M=64, N=256]
    (4 heads per matmul) instead of 64 of [K=32, M=64, N=64].
  - exp via ACT (scale folded); multiply by resident EB table
    (gpsimd/vector alternating).
  - V projection (+bv via ones-matmul); vswap partition-swap DMA; V
    scattered into block-diag augmented va_all (ones cols for softmax
    sums) with 4 bulk strided copies per chunk.
  - ctx: 8 matmuls/pair [K=128, M=64, N=66] pairing heads (j, j+4);
    DVE reciprocal + broadcast multiply normalizes into bf16 staging;
    1 output DMA per chunk.
"""

import numpy as np
import ml_dtypes

import concourse.bass as bass
import concourse.mybir as mybir
import concourse.tile as tile
from concourse.bass_utils import run_bass_kernel_spmd

DT = mybir.dt
F32 = DT.float32
BF16 = DT.bfloat16

N_CORES = 8
B = 2048
C = 256
H = 8
D = 32
WINTOK = 64
SCALE = 1.0 / np.sqrt(np.float32(D))


def _wait_cap(inst):
    """Max sem waits the walrus encoding of this instruction tolerates."""
    if isinstance(inst, (mybir.InstDrain, mybir.InstNoOp)):
        return 1  # CTRL_NO_STRUCT
    if isinstance(inst, (mybir.InstDMACopy, mybir.InstDMA, mybir.InstDmaTransposeAnt)):
        return 1  # PSEUDO_DMA_DIRECT2D
    return 1


def split_drain_waits(nc):
    """Walrus instruction encodings only fit a limited number of sem waits;
    Tile can attach more. Hoist excess waits onto NoOps inserted before the
    instruction on the same engine (in-order sequencers make this
    equivalent, if slightly more conservative)."""
    for f in nc.m.functions:
        for bb in f.blocks:
            new_insts = []
            for inst in bb.instructions:
                si = inst.sync_info
                cap = _wait_cap(inst)
                if si is not None and si.on_wait and len(si.on_wait) > cap:
                    waits = list(si.on_wait)
                    keep, rest = waits[:cap], waits[cap:]
                    for i in range(0, len(rest), 1):
                        new_insts.append(
                            mybir.InstNoOp(
                                name=f"{inst.name}-waitsplit-{i}",
                                engine=inst.engine,
                                sync_info=mybir.SyncInfo(
                                    on_wait=[rest[i]], on_update=[]
                                ),
                            )
                        )
                    inst.sync_info = mybir.SyncInfo(
                        on_wait=keep, on_update=list(si.on_update or [])
                    )
                new_insts.append(inst)
            bb.instructions[:] = new_insts


def build(n_windows=256, chunk_windows=8, split_waits=True):
    """Emit the per-core kernel.

    DRAM layouts:
      xt   [256, ntok] bf16 (host-transposed hidden states)
      eb   [32, 128, 512] bf16:
           eb[t, g*64+k, win*256+hh*64+q] =
               exp(rpb[g*4+hh, q, k] + mask[(2t+win)%64, q, k])
      wqt/wkt/wvt [256, 256] bf16 = W.T  ([C_in, C_out])
      bqv/bkv DRAM [256] f32 ; bvr DRAM [128, 256] bf16 (bv broadcast)
      out  [ntok, 256] bf16
    """
    assert n_windows % chunk_windows == 0 and chunk_windows % 2 == 0
    ntok = n_windows * WINTOK
    n_chunks = n_windows // chunk_windows
    chunk_tok = chunk_windows * WINTOK  # 512
    n_pairs = chunk_windows // 2  # 4 window pairs per chunk
    assert chunk_tok == 512

    nc = bass.Bass()
    xt = nc.declare_dram_parameter("xt", [C, ntok], BF16, isOutput=False)
    eb = nc.declare_dram_parameter("eb", [32, 128, 512], BF16, isOutput=False)
    wqt = nc.declare_dram_parameter("wqt", [C, C], BF16, isOutput=False)
    wkt = nc.declare_dram_parameter("wkt", [C, C], BF16, isOutput=False)
    wvt = nc.declare_dram_parameter("wvt", [C, C], BF16, isOutput=False)
    bqv = nc.declare_dram_parameter("bqv", [C], F32, isOutput=False)
    bkv = nc.declare_dram_parameter("bkv", [C], F32, isOutput=False)
    bvr = nc.declare_dram_parameter("bvr", [128, C], BF16, isOutput=False)
    out = nc.declare_dram_parameter("out", [ntok, C], BF16, isOutput=True)

    with tile.TileContext(nc) as tc:
        with (
            tc.tile_pool(name="const", bufs=1) as cpool,
            tc.tile_pool(name="acts", bufs=2) as apool,
            tc.tile_pool(name="probs", bufs=3) as ppool,
            tc.tile_pool(name="stage", bufs=2) as spool,
            tc.tile_pool(name="small", bufs=3) as smpool,
            tc.tile_pool(name="vv", bufs=2) as vpool,
            tc.tile_pool(name="ppj", bufs=2, space="PSUM") as ppj,
            tc.tile_pool(name="ppv", bufs=2, space="PSUM") as ppv,
            tc.tile_pool(name="psc", bufs=2, space="PSUM") as psc,
            tc.tile_pool(name="pctx", bufs=2, space="PSUM") as pctx,
        ):
            # ---- constants ----
            eb_sb = cpool.tile([128, 32 * 512], BF16)
            nc.sync.dma_start(
                eb_sb.rearrange("p (t c) -> p t c", t=32),
                eb.rearrange("t p c -> p t c"),
            )
            wq_sb = cpool.tile([128, 512], BF16)
            wk_sb = cpool.tile([128, 512], BF16)
            wv_sb = cpool.tile([128, 512], BF16)
            for w_sb, w_dram in ((wq_sb, wqt), (wk_sb, wkt), (wv_sb, wvt)):
                nc.sync.dma_start(
                    w_sb.rearrange("p (ck c) -> p ck c", ck=2),
                    w_dram.rearrange("(ck p) c -> p ck c", p=128),
                )
            bq_sb = cpool.tile([128, 2], F32)
            bk_sb = cpool.tile([128, 2], F32)
            nc.sync.dma_start(bq_sb[:], bqv.rearrange("(m p) -> p m", p=128))
            nc.sync.dma_start(bk_sb[:], bkv.rearrange("(m p) -> p m", p=128))
            # bv as K=128 constant matmul: lhsT = 1/128, rhs = bv broadcast
            inv_sb = cpool.tile([128, 128], BF16)
            nc.vector.memset(inv_sb[:], 1.0 / 128.0)
            bvb_sb = cpool.tile([128, C], BF16)
            nc.sync.dma_start(bvb_sb[:], bvr[:, :])

            # qbd: block-diag Q operand, double buffered; zeros are static.
            # qbd[32r:32r+32, (g, w, hh, q)] = Q^T band r iff hh == r.
            qbds = []
            for ub in range(2):
                qb = cpool.tile([128, 2 * 8 * 256], BF16, tag=f"qbd{ub}")
                nc.vector.memset(qb[:], 0.0)
                qbds.append(qb)

            # va_all: per chunk [128, 8 windows x 264]; block j of window w:
            # rows 0:63  cols j*66+0:32  = V_{j}(w)[k, d],   col j*66+32 ones
            # rows 64:128 cols j*66+33:65 = V_{j+4}(w)[k, d], col j*66+65 ones
            vas = []
            for ub in range(2):
                va = cpool.tile([128, 8 * 264], BF16, tag=f"vaall{ub}")
                nc.vector.memset(va[:], 0.0)
                nc.vector.memset(
                    va[0:64, :].rearrange("p (w j c) -> p w j c", w=8, j=4)[
                        :, :, :, 32:33
                    ],
                    1.0,
                )
                nc.vector.memset(
                    va[64:128, :].rearrange("p (w j c) -> p w j c", w=8, j=4)[
                        :, :, :, 65:66
                    ],
                    1.0,
                )
                vas.append(va)

            # ---- main loop over chunks ----
            for u in range(n_chunks):
                t0 = u * chunk_tok
                qbd = qbds[u % 2]
                va = vas[u % 2]

                # X^T chunk: plain loads (host already transposed)
                xt_sb = apool.tile([128, 2 * 512], BF16, tag="xt")
                for ck in range(2):
                    nc.sync.dma_start(
                        xt_sb[:, ck * 512 : (ck + 1) * 512],
                        xt[ck * 128 : (ck + 1) * 128, t0 : t0 + chunk_tok],
                    )

                # Q^T / K^T projections (weights stationary)
                qt_sb = apool.tile([128, 2 * 512], BF16, tag="qt")
                kt_sb = apool.tile([128, 2 * 512], BF16, tag="kt")
                for qk, (w_sb, b_sb, dst) in enumerate(
                    ((wq_sb, bq_sb, qt_sb), (wk_sb, bk_sb, kt_sb))
                ):
                    for m in range(2):
                        prj = ppj.tile([128, 512], F32, tag="ppj")
                        for ck in range(2):
                            nc.tensor.matmul(
                                prj[:],
                                w_sb[:, ck * 256 + m * 128 : ck * 256 + (m + 1) * 128],
                                xt_sb[:, ck * 512 : (ck + 1) * 512],
                                start=(ck == 0),
                                stop=(ck == 1),
                            )
                        cp_out = dst[:, m * 512 : (m + 1) * 512]
                        if qk == 0:
                            nc.scalar.activation(
                                cp_out,
                                prj[:],
                                mybir.ActivationFunctionType.Identity,
                                bias=b_sb[:, m : m + 1],
                            )
                        else:
                            nc.vector.tensor_scalar_add(
                                cp_out, prj[:], b_sb[:, m : m + 1]
                            )

                # qbd build: one SB->SB DMA per band r (data regions only;
                # zeros are static). Issued on the scalar HWDGE ring.
                for r in range(4):
                    nc.scalar.dma_start(
                        qbd[32 * r : 32 * r + 32, :].rearrange(
                            "p (g w hh q) -> p g w hh q", g=2, w=8, hh=4
                        )[:, :, :, r, :],
                        qt_sb[32 * r : 32 * r + 32, :].rearrange(
                            "p (g w q) -> p g w q", g=2, w=8
                        ),
                    )

                # V projection + bv
                vtmp = vpool.tile([128, 4 * 256], BF16, tag="vt")
                for i in range(4):
                    vps = ppv.tile([128, 256], F32, tag="ppv")
                    for ck in range(2):
                        nc.tensor.matmul(
                            vps[:],
                            xt_sb[:, ck * 512 + i * 128 : ck * 512 + (i + 1) * 128],
                            wv_sb[:, ck * 256 : (ck + 1) * 256],
                            start=(ck == 0),
                            stop=False,
                        )
                    nc.tensor.matmul(
                        vps[:], inv_sb[:, :], bvb_sb[:, :], start=False, stop=True
                    )
                    nc.vector.tensor_copy(
                        vtmp[:, i * 256 : (i + 1) * 256], vps[:]
                    )
                vswap = vpool.tile([128, 4 * 256], BF16, tag="vs")
                nc.sync.dma_start(vswap[64:128, :], vtmp[0:64, :])
                nc.sync.dma_start(vswap[0:64, :], vtmp[64:128, :])

                # va_all bulk fills (4 strided copies per chunk)
                va_top = va[0:64, :].rearrange(
                    "p (i win j c) -> p i win j c", i=4, win=2, j=4
                )
                va_bot = va[64:128, :].rearrange(
                    "p (i win j c) -> p i win j c", i=4, win=2, j=4
                )
                vt_top = vtmp[0:64, :].rearrange(
                    "p (i m j d) -> p i m j d", i=4, m=2, j=4
                )
                vt_bot = vtmp[64:128, :].rearrange(
                    "p (i m j d) -> p i m j d", i=4, m=2, j=4
                )
                vs_top = vswap[0:64, :].rearrange(
                    "p (i m j d) -> p i m j d", i=4, m=2, j=4
                )
                vs_bot = vswap[64:128, :].rearrange(
                    "p (i m j d) -> p i m j d", i=4, m=2, j=4
                )
                # win0 tops from vtmp, win1 tops from vswap
                nc.vector.tensor_copy(
                    va_top[:, :, 0, :, 0:32], vt_top[:, :, 0, :, :]
                )
                nc.scalar.copy(va_top[:, :, 1, :, 0:32], vs_top[:, :, 0, :, :])
                # win0 bottoms from vswap (m1), win1 bottoms from vtmp (m1)
                nc.scalar.copy(va_bot[:, :, 0, :, 33:65], vs_bot[:, :, 1, :, :])
                nc.vector.tensor_copy(
                    va_bot[:, :, 1, :, 33:65], vt_bot[:, :, 1, :, :]
                )

                stgc = spool.tile([128, 4 * 256], BF16, tag="st")
                for i in range(n_pairs):
                    # ---- scores: 4 matmuls [K=128, M=64, N=256] ----
                    scp = psc.tile([128, 512], F32, tag="sc")
                    for g in range(2):
                        for win in range(2):
                            w_local = i * 2 + win
                            nc.tensor.matmul(
                                scp[g * 64 : g * 64 + 64, win * 256 : win * 256 + 256],
                                kt_sb[
                                    :,
                                    g * 512 + w_local * 64 : g * 512 + w_local * 64 + 64,
                                ],
                                qbd[:, (g * 8 + w_local) * 256 : (g * 8 + w_local + 1) * 256],
                                start=True,
                                stop=True,
                                tile_position=(0, g * 64),
                            )
                    probs = ppool.tile([128, 512], BF16, tag="pr")
                    nc.scalar.activation(
                        probs[:],
                        scp[:],
                        mybir.ActivationFunctionType.Exp,
                        scale=float(SCALE),
                    )
                    t_slot = (u * n_pairs + i) % 32
                    ebs = eb_sb[:, t_slot * 512 : (t_slot + 1) * 512]
                    if i % 2 == 0:
                        nc.gpsimd.tensor_mul(probs[:], probs[:], ebs)
                    else:
                        nc.vector.tensor_mul(probs[:], probs[:], ebs)

                    # ---- ctx: 8 matmuls [K=128, M=64, N=66] ----
                    ctxp = pctx.tile([128, 264], F32, tag="ctx")
                    for win in range(2):
                        for j in range(4):
                            nc.tensor.matmul(
                                ctxp[win * 64 : win * 64 + 64, j * 66 : j * 66 + 66],
                                probs[:, win * 256 + j * 64 : win * 256 + j * 64 + 64],
                                va[:, (i * 2 + win) * 264 + j * 66 : (i * 2 + win) * 264 + j * 66 + 66],
                                start=True,
                                stop=True,
                                tile_position=(0, win * 64),
                            )

                    recips = smpool.tile([128, 8], F32, tag="rc")
                    sums_ap = ctxp.rearrange("p (j par c) -> p j par c", j=4, par=2)[
                        :, :, :, 32:33
                    ]
                    nc.vector.reciprocal(recips[:], sums_ap)

                    # normalize into stgc, cols ordered (par, j, d) so the
                    # output DMA is contiguous per token
                    ctx_ap = ctxp.rearrange("p (j par c) -> p par j c", j=4, par=2)[
                        :, :, :, 0:32
                    ]
                    rec_ap = recips.rearrange(
                        "p (j par one) -> p par j one", j=4, par=2, one=1
                    )
                    ctx_b, rec_b = bass.broadcast_tensor_aps(ctx_ap, rec_ap)
                    out_ap = stgc[:, i * 256 : (i + 1) * 256].rearrange(
                        "p (par j c) -> p par j c", par=2, j=4
                    )
                    nc.vector.tensor_tensor(out_ap, ctx_b, rec_b, mybir.AluOpType.mult)

                nc.sync.dma_start(
                    out[t0 : t0 + chunk_tok, :].rearrange("(i p) c -> p i c", p=128),
                    stgc.rearrange("p (i c) -> p i c", i=4),
                )

    if split_waits:
        split_drain_waits(nc)
    return nc


def ref_shard(x, eb_full, wq, bq, wk, bk, wv, bv):
    """NumPy reference for one shard (dev-time check)."""
    ntok = x.shape[0]
    nwin = ntok // WINTOK
    q = x @ wq.T + bq
    k = x @ wk.T + bk
    v = x @ wv.T + bv
    outp = np.zeros((ntok, C), np.float32)
    for w in range(nwin):
        t = slice(w * WINTOK, (w + 1) * WINTOK)
        for h in range(H):
            qh = q[t, h * D : (h + 1) * D]
            kh = k[t, h * D : (h + 1) * D]
            vh = v[t, h * D : (h + 1) * D]
            sT = (kh @ qh.T) * SCALE
            g, hh = h // 4, h % 4
            tp, win = (w // 2) % 32, w % 2
            ebm = eb_full[tp, g * 64 : g * 64 + 64, win * 256 + hh * 64 : win * 256 + hh * 64 + 64]
            pu = np.exp(sT) * ebm
            ctx = pu.T @ vh
            s = pu.sum(axis=0)
            outp[t, h * D : (h + 1) * D] = ctx / s[:, None]
    return outp


_NC_CACHE = {}


def _get_nc():
    key = "main"
    if key not in _NC_CACHE:
        _NC_CACHE[key] = build(n_windows=B // N_CORES)
    return _NC_CACHE[key]


def _pack_eb(bias_table, rel_index, attention_mask):
    # rpb[h, q, k] = bias_table[rel_index[q, k], h]
    rpb = bias_table[rel_index.reshape(-1)].reshape(64, 64, H).transpose(2, 0, 1)
    e = np.exp(
        rpb[None].astype(np.float64) + attention_mask[:, None].astype(np.float64)
    ).astype(np.float32)
    # e [nw, h, q, k] -> eb[t, g*64 + k, win*256 + hh*64 + q]
    # where nw = 2t + win, h = g*4 + hh
    e2 = e.transpose(0, 1, 3, 2)  # [nw, h, k, q]
    e3 = e2.reshape(32, 2, 2, 4, 64, 64)  # [t, win, g, hh, k, q]
    e4 = e3.transpose(0, 2, 4, 1, 3, 5)  # [t, g, k, win, hh, q]
    return np.ascontiguousarray(e4.reshape(32, 128, 512))


def build_in_maps(
    hidden_states,
    attention_mask,
    Wq,
    bq,
    Wk,
    bk,
    Wv,
    bv,
    bias_table,
    rel_index,
):
    bf = ml_dtypes.bfloat16
    xs = np.ascontiguousarray(
        np.asarray(hidden_states, np.float32).reshape(B * WINTOK, C).T
    ).astype(bf)
    eb = _pack_eb(
        np.asarray(bias_table, np.float32),
        np.asarray(rel_index),
        np.asarray(attention_mask, np.float32),
    ).astype(bf)
    common = {
        "eb": eb,
        "wqt": np.ascontiguousarray(Wq.T).astype(bf),
        "wkt": np.ascontiguousarray(Wk.T).astype(bf),
        "wvt": np.ascontiguousarray(Wv.T).astype(bf),
        "bqv": np.asarray(bq, np.float32),
        "bkv": np.asarray(bk, np.float32),
        "bvr": np.tile(np.asarray(bv, np.float32)[None, :], (128, 1)).astype(bf),
    }
    shard_tok = (B // N_CORES) * WINTOK
    return [
        {"xt": np.ascontiguousarray(xs[:, c * shard_tok : (c + 1) * shard_tok]), **common}
        for c in range(N_CORES)
    ]


def kernel(
    hidden_states,
    attention_mask,
    Wq,
    bq,
    Wk,
    bk,
    Wv,
    bv,
    bias_table,
    rel_index,
):
    nc = _get_nc()
    in_maps = build_in_maps(
        hidden_states, attention_mask, Wq, bq, Wk, bk, Wv, bv, bias_table, rel_index
    )
    res = run_bass_kernel_spmd(nc, in_maps, list(range(N_CORES)))
    outp = np.concatenate(
        [res.results[c]["out"] for c in range(N_CORES)], axis=0
    )
    return outp.reshape(B, WINTOK, C).astype(np.float32)


# revision 9
# speedup vs baseline: 1.3235x; 1.1985x over previous
"""TRN2 Bass kernel: ClapAudio window self-attention (B=2048 windows of 64
tokens, C=256, 8 heads x d=32), data-parallel over windows across 8 cores.

Host side: shards + pre-transposes hidden_states (xt [C, ntok]), precomputes
EB = exp(rel-pos-bias + mask)^T (folding both additive score biases into one
multiplicative table applied after exp), passes transposed weights in bf16.
Output returned bf16 from device, cast to f32 on host.

Device side (per core, 256 windows, 32 chunks of 8 windows), software
pipelined one chunk deep:  scores(u) -> [prep u+1: proj/qbd/V/va] -> ctx(u).
  - qbd: block-diagonal Q operand, band-contiguous layout (hh, g, w, q):
    band r of Q^T lands in col block hh=r via a trivial [32, 1024]
    contiguous SB->SB DMA; static zeros elsewhere. Scores take 16
    matmuls/chunk of [K=128, M=64, N=256] (4 heads per matmul, strided rhs).
  - exp via ACT (scale folded); EB multiply on gpsimd.
  - V projection; bv fused into the PSUM->SBUF cast (tensor_tensor add);
    V scattered into block-diag augmented va (ones cols for softmax sums)
    by 2 DVE copies + 2 partition-swapping strided SB->SB DMAs.
  - ctx: 8 matmuls/pair [K=128, M=64, N=66] pairing heads (j, j+4);
    DVE reciprocal + broadcast multiply normalizes into bf16 staging;
    1 output DMA per chunk.
"""

import numpy as np
import ml_dtypes

import concourse.bass as bass
import concourse.mybir as mybir
import concourse.tile as tile
from concourse.bass_utils import run_bass_kernel_spmd

DT = mybir.dt
F32 = DT.float32
BF16 = DT.bfloat16

N_CORES = 8
B = 2048
C = 256
H = 8
D = 32
WINTOK = 64
SCALE = 1.0 / np.sqrt(np.float32(D))


def _wait_cap(inst):
    """Max sem waits the walrus encoding of this instruction tolerates."""
    if isinstance(inst, (mybir.InstDrain, mybir.InstNoOp)):
        return 1  # CTRL_NO_STRUCT
    if isinstance(inst, (mybir.InstDMACopy, mybir.InstDMA, mybir.InstDmaTransposeAnt)):
        return 1  # PSEUDO_DMA_DIRECT2D
    return 1


def split_drain_waits(nc):
    """Walrus instruction encodings only fit a limited number of sem waits;
    Tile can attach more. Hoist excess waits onto NoOps inserted before the
    instruction on the same engine (in-order sequencers make this
    equivalent, if slightly more conservative)."""
    for f in nc.m.functions:
        for bb in f.blocks:
            new_insts = []
            for inst in bb.instructions:
                si = inst.sync_info
                cap = _wait_cap(inst)
                if si is not None and si.on_wait and len(si.on_wait) > cap:
                    waits = list(si.on_wait)
                    keep, rest = waits[:cap], waits[cap:]
                    for i in range(0, len(rest), 1):
                        new_insts.append(
                            mybir.InstNoOp(
                                name=f"{inst.name}-waitsplit-{i}",
                                engine=inst.engine,
                                sync_info=mybir.SyncInfo(
                                    on_wait=[rest[i]], on_update=[]
                                ),
                            )
                        )
                    inst.sync_info = mybir.SyncInfo(
                        on_wait=keep, on_update=list(si.on_update or [])
                    )
                new_insts.append(inst)
            bb.instructions[:] = new_insts


def build(n_windows=256, chunk_windows=8, split_waits=True):
    """Emit the per-core kernel.

    DRAM layouts:
      xt   [256, ntok] bf16 (host-transposed hidden states)
      eb   [32, 128, 512] bf16:
           eb[t, g*64+k, win*256+hh*64+q] =
               exp(rpb[g*4+hh, q, k] + mask[(2t+win)%64, q, k])
      wqt/wkt/wvt [256, 256] bf16 = W.T  ([C_in, C_out])
      bqv/bkv DRAM [256] f32 ; bvr DRAM [128, 256] bf16 (bv broadcast)
      out  [ntok, 256] bf16
    """
    assert n_windows % chunk_windows == 0 and chunk_windows % 2 == 0
    ntok = n_windows * WINTOK
    n_chunks = n_windows // chunk_windows
    chunk_tok = chunk_windows * WINTOK  # 512
    n_pairs = chunk_windows // 2  # 4 window pairs per chunk
    assert chunk_tok == 512

    nc = bass.Bass()
    xt = nc.declare_dram_parameter("xt", [C, ntok], BF16, isOutput=False)
    eb = nc.declare_dram_parameter("eb", [32, 128, 512], BF16, isOutput=False)
    wqt = nc.declare_dram_parameter("wqt", [C, C], BF16, isOutput=False)
    wkt = nc.declare_dram_parameter("wkt", [C, C], BF16, isOutput=False)
    wvt = nc.declare_dram_parameter("wvt", [C, C], BF16, isOutput=False)
    bqv = nc.declare_dram_parameter("bqv", [C], F32, isOutput=False)
    bkv = nc.declare_dram_parameter("bkv", [C], F32, isOutput=False)
    bvr = nc.declare_dram_parameter("bvr", [128, C], BF16, isOutput=False)
    out = nc.declare_dram_parameter("out", [ntok, C], BF16, isOutput=True)

    with tile.TileContext(nc) as tc:
        with (
            tc.tile_pool(name="const", bufs=1) as cpool,
            tc.tile_pool(name="acts", bufs=2) as apool,
            tc.tile_pool(name="probs", bufs=5) as ppool,
            tc.tile_pool(name="stage", bufs=2) as spool,
            tc.tile_pool(name="small", bufs=4) as smpool,
            tc.tile_pool(name="vv", bufs=2) as vpool,
            tc.tile_pool(name="ppj", bufs=2, space="PSUM") as ppj,
            tc.tile_pool(name="ppv", bufs=1, space="PSUM") as ppv,
            tc.tile_pool(name="psc", bufs=3, space="PSUM") as psc,
            tc.tile_pool(name="pctx", bufs=2, space="PSUM") as pctx,
        ):
            # ---- constants ----
            eb_sb = cpool.tile([128, 32 * 512], BF16)
            nc.sync.dma_start(
                eb_sb.rearrange("p (t c) -> p t c", t=32),
                eb.rearrange("t p c -> p t c"),
            )
            wq_sb = cpool.tile([128, 512], BF16)
            wk_sb = cpool.tile([128, 512], BF16)
            wv_sb = cpool.tile([128, 512], BF16)
            for w_sb, w_dram in ((wq_sb, wqt), (wk_sb, wkt), (wv_sb, wvt)):
                nc.sync.dma_start(
                    w_sb.rearrange("p (ck c) -> p ck c", ck=2),
                    w_dram.rearrange("(ck p) c -> p ck c", p=128),
                )
            bq_sb = cpool.tile([128, 2], F32)
            bk_sb = cpool.tile([128, 2], F32)
            nc.sync.dma_start(bq_sb[:], bqv.rearrange("(m p) -> p m", p=128))
            nc.sync.dma_start(bk_sb[:], bkv.rearrange("(m p) -> p m", p=128))
            bvb_sb = cpool.tile([128, C], BF16)
            nc.sync.dma_start(bvb_sb[:], bvr[:, :])

            # qbd: block-diag Q operand, band-contiguous (hh, g, w, q);
            # double buffered, zeros static.
            qbds = []
            for ub in range(2):
                qb = cpool.tile([128, 4 * 1024], BF16, tag=f"qbd{ub}")
                nc.vector.memset(qb[:], 0.0)
                qbds.append(qb)

            # va: per chunk [128, (win, i, j, 66)]; block (win, i, j):
            # rows 0:64  cols +0:32  = V_{j}(w)[k, d],   col +32 ones
            # rows 64:128 cols +33:65 = V_{j+4}(w)[k, d], col +65 ones
            # (win-major layout so swap-DMA dest APs collapse to 3 dims)
            vas = []
            for ub in range(2):
                va = cpool.tile([128, 8 * 264], BF16, tag=f"vaall{ub}")
                nc.vector.memset(va[:], 0.0)
                nc.vector.memset(
                    va[0:64, :].rearrange("p (w j c) -> p w j c", w=8, j=4)[
                        :, :, :, 32:33
                    ],
                    1.0,
                )
                nc.vector.memset(
                    va[64:128, :].rearrange("p (w j c) -> p w j c", w=8, j=4)[
                        :, :, :, 65:66
                    ],
                    1.0,
                )
                vas.append(va)

            def load_xt(u):
                xt_sb = apool.tile([128, 2 * 512], BF16, tag="xt")
                t0 = u * chunk_tok
                nc.sync.dma_start(
                    xt_sb.rearrange("p (ck t) -> p ck t", ck=2),
                    xt.rearrange("(ck p) t -> p ck t", p=128)[
                        :, :, t0 : t0 + chunk_tok
                    ],
                )
                return xt_sb

            def prep(u, xt_sb):
                """Projections + qbd + V + va for chunk u."""
                qbd = qbds[u % 2]
                va = vas[u % 2]

                qt_sb = apool.tile([128, 2 * 512], BF16, tag="qt")
                kt_sb = apool.tile([128, 2 * 512], BF16, tag="kt")
                for qk, (w_sb, b_sb, dst) in enumerate(
                    ((wq_sb, bq_sb, qt_sb), (wk_sb, bk_sb, kt_sb))
                ):
                    for m in range(2):
                        prj = ppj.tile([128, 512], F32, tag="ppj")
                        for ck in range(2):
                            nc.tensor.matmul(
                                prj[:],
                                w_sb[:, ck * 256 + m * 128 : ck * 256 + (m + 1) * 128],
                                xt_sb[:, ck * 512 : (ck + 1) * 512],
                                start=(ck == 0),
                                stop=(ck == 1),
                            )
                        cp_out = dst[:, m * 512 : (m + 1) * 512]
                        if qk == 0:
                            nc.scalar.activation(
                                cp_out,
                                prj[:],
                                mybir.ActivationFunctionType.Identity,
                                bias=b_sb[:, m : m + 1],
                            )
                        else:
                            nc.vector.tensor_scalar_add(
                                cp_out, prj[:], b_sb[:, m : m + 1]
                            )

                # qbd band DMAs: contiguous [32, 1024] each; 2 on the sync
                # HWDGE ring, 2 on the scalar ring.
                for r in range(4):
                    eng = nc.sync if r % 2 == 0 else nc.scalar
                    eng.dma_start(
                        qbd[32 * r : 32 * r + 32, r * 1024 : (r + 1) * 1024],
                        qt_sb[32 * r : 32 * r + 32, :],
                    )

                # V projection; bv fused into the cast. vtmp cols are
                # (m, i, j, d) so each swap-DMA source is one contiguous run.
                vtmp = vpool.tile([128, 4 * 256], BF16, tag="vt")
                for i in range(4):
                    vps = ppv.tile([128, 256], F32, tag="ppv")
                    for ck in range(2):
                        nc.tensor.matmul(
                            vps[:],
                            xt_sb[:, ck * 512 + i * 128 : ck * 512 + (i + 1) * 128],
                            wv_sb[:, ck * 256 : (ck + 1) * 256],
                            start=(ck == 0),
                            stop=(ck == 1),
                        )
                    nc.vector.tensor_tensor(
                        vtmp.rearrange("p (m i jd) -> p m i jd", m=2, i=4)[:, :, i, :],
                        vps.rearrange("p (m jd) -> p m jd", m=2),
                        bvb_sb.rearrange("p (m jd) -> p m jd", m=2),
                        mybir.AluOpType.add,
                    )

                # va cols: (win, i, j, 66). vtmp cols: (m, i, j, d).
                va_top = va[0:64, :].rearrange(
                    "p (win i j c) -> p win i j c", win=2, i=4, j=4
                )
                va_bot = va[64:128, :].rearrange(
                    "p (win i j c) -> p win i j c", win=2, i=4, j=4
                )
                # partition-swapping fills via SB->SB DMA (3-dim APs)
                nc.sync.dma_start(
                    va_top[:, 1, :, :, 0:32], vtmp[64:128, 0:512]
                )
                nc.sync.dma_start(
                    va_bot[:, 0, :, :, 33:65], vtmp[0:64, 512:1024]
                )
                # same-partition fills on DVE
                nc.vector.tensor_copy(
                    va_top[:, 0, :, :, 0:32],
                    vtmp[0:64, 0:512].rearrange("p (i j d) -> p i j d", i=4, j=4),
                )
                nc.vector.tensor_copy(
                    va_bot[:, 1, :, :, 33:65],
                    vtmp[64:128, 512:1024].rearrange("p (i j d) -> p i j d", i=4, j=4),
                )
                return qt_sb, kt_sb, qbd, va

            def scores(u, kt_sb, qbd):
                """Score matmuls + exp + EB multiply; returns probs tiles."""
                qbd_v = qbd.rearrange("p (hh g w q) -> p hh g w q", hh=4, g=2, w=8)
                probs_l = []
                for i in range(n_pairs):
                    scp = psc.tile([128, 512], F32, tag="sc")
                    for g in range(2):
                        for win in range(2):
                            wl = i * 2 + win
                            nc.tensor.matmul(
                                scp[g * 64 : g * 64 + 64, win * 256 : win * 256 + 256],
                                kt_sb[:, g * 512 + wl * 64 : g * 512 + wl * 64 + 64],
                                qbd_v[:, :, g, wl, :],
                                start=True,
                                stop=True,
                                tile_position=(0, g * 64),
                            )
                    probs = ppool.tile([128, 512], BF16, tag="pr")
                    nc.scalar.activation(
                        probs[:],
                        scp[:],
                        mybir.ActivationFunctionType.Exp,
                        scale=float(SCALE),
                    )
                    t_slot = (u * n_pairs + i) % 32
                    ebs = eb_sb[:, t_slot * 512 : (t_slot + 1) * 512]
                    nc.gpsimd.tensor_mul(probs[:], probs[:], ebs)
                    probs_l.append(probs)
                return probs_l

            def ctx_out(u, probs_l, va):
                stgc = spool.tile([128, 4 * 256], BF16, tag="st")
                for i in range(n_pairs):
                    probs = probs_l[i]
                    ctxp = pctx.tile([128, 264], F32, tag="ctx")
                    for win in range(2):
                        for j in range(4):
                            nc.tensor.matmul(
                                ctxp[win * 64 : win * 64 + 64, j * 66 : j * 66 + 66],
                                probs[:, win * 256 + j * 64 : win * 256 + j * 64 + 64],
                                va[
                                    :,
                                    win * 1056 + i * 264 + j * 66 : win * 1056
                                    + i * 264
                                    + j * 66
                                    + 66,
                                ],
                                start=True,
                                stop=True,
                                tile_position=(0, win * 64),
                            )
                    recips = smpool.tile([128, 8], F32, tag="rc")
                    sums_ap = ctxp.rearrange("p (j par c) -> p j par c", j=4, par=2)[
                        :, :, :, 32:33
                    ]
                    nc.vector.reciprocal(recips[:], sums_ap)
                    ctx_ap = ctxp.rearrange("p (j par c) -> p par j c", j=4, par=2)[
                        :, :, :, 0:32
                    ]
                    rec_ap = recips.rearrange(
                        "p (j par one) -> p par j one", j=4, par=2, one=1
                    )
                    ctx_b, rec_b = bass.broadcast_tensor_aps(ctx_ap, rec_ap)
                    out_ap = stgc[:, i * 256 : (i + 1) * 256].rearrange(
                        "p (par j c) -> p par j c", par=2, j=4
                    )
                    nc.vector.tensor_tensor(out_ap, ctx_b, rec_b, mybir.AluOpType.mult)
                t0 = u * chunk_tok
                nc.sync.dma_start(
                    out[t0 : t0 + chunk_tok, :].rearrange("(i p) c -> p i c", p=128),
                    stgc.rearrange("p (i c) -> p i c", i=4),
                )

            # ---- software-pipelined main loop ----
            xt_cur = load_xt(0)
            qt_sb, kt_sb, qbd, va = prep(0, xt_cur)
            for u in range(n_chunks):
                if u + 1 < n_chunks:
                    xt_nxt = load_xt(u + 1)
                probs_l = scores(u, kt_sb, qbd)
                if u + 1 < n_chunks:
                    nxt = prep(u + 1, xt_nxt)
                ctx_out(u, probs_l, va)
                if u + 1 < n_chunks:
                    qt_sb, kt_sb, qbd, va = nxt

    if split_waits:
        split_drain_waits(nc)
    return nc


_NC_CACHE = {}


def _get_nc():
    key = "main"
    if key not in _NC_CACHE:
        _NC_CACHE[key] = build(n_windows=B // N_CORES)
    return _NC_CACHE[key]


def _pack_eb(bias_table, rel_index, attention_mask):
    # rpb[h, q, k] = bias_table[rel_index[q, k], h]
    rpb = bias_table[rel_index.reshape(-1)].reshape(64, 64, H).transpose(2, 0, 1)
    e = np.exp(
        rpb[None].astype(np.float64) + attention_mask[:, None].astype(np.float64)
    ).astype(np.float32)
    # e [nw, h, q, k] -> eb[t, g*64 + k, win*256 + hh*64 + q]
    # where nw = 2t + win, h = g*4 + hh
    e2 = e.transpose(0, 1, 3, 2)  # [nw, h, k, q]
    e3 = e2.reshape(32, 2, 2, 4, 64, 64)  # [t, win, g, hh, k, q]
    e4 = e3.transpose(0, 2, 4, 1, 3, 5)  # [t, g, k, win, hh, q]
    return np.ascontiguousarray(e4.reshape(32, 128, 512))


def build_in_maps(
    hidden_states,
    attention_mask,
    Wq,
    bq,
    Wk,
    bk,
    Wv,
    bv,
    bias_table,
    rel_index,
):
    bf = ml_dtypes.bfloat16
    xs = np.ascontiguousarray(
        np.asarray(hidden_states, np.float32).reshape(B * WINTOK, C).T
    ).astype(bf)
    eb = _pack_eb(
        np.asarray(bias_table, np.float32),
        np.asarray(rel_index),
        np.asarray(attention_mask, np.float32),
    ).astype(bf)
    common = {
        "eb": eb,
        "wqt": np.ascontiguousarray(Wq.T).astype(bf),
        "wkt": np.ascontiguousarray(Wk.T).astype(bf),
        "wvt": np.ascontiguousarray(Wv.T).astype(bf),
        "bqv": np.asarray(bq, np.float32),
        "bkv": np.asarray(bk, np.float32),
        "bvr": np.tile(np.asarray(bv, np.float32)[None, :], (128, 1)).astype(bf),
    }
    shard_tok = (B // N_CORES) * WINTOK
    return [
        {"xt": np.ascontiguousarray(xs[:, c * shard_tok : (c + 1) * shard_tok]), **common}
        for c in range(N_CORES)
    ]


def kernel(
    hidden_states,
    attention_mask,
    Wq,
    bq,
    Wk,
    bk,
    Wv,
    bv,
    bias_table,
    rel_index,
):
    nc = _get_nc()
    in_maps = build_in_maps(
        hidden_states, attention_mask, Wq, bq, Wk, bk, Wv, bv, bias_table, rel_index
    )
    res = run_bass_kernel_spmd(nc, in_maps, list(range(N_CORES)))
    outp = np.concatenate(
        [res.results[c]["out"] for c in range(N_CORES)], axis=0
    )
    return outp.reshape(B, WINTOK, C).astype(np.float32)
